# revision 20
# baseline (speedup 1.0000x reference)
"""Trainium2 Bass kernel for nn_CycleGNN (8-step projected-direction solver).

Contract: kernel(**inputs) takes the FULL unsharded numpy inputs (keyed as in
setup_inputs()) and returns the full output (preds, labels), each
[131072, 8] float32.  Internally shards the 64 graphs across 8 NeuronCores
(8 graphs per core, graphs never interact -> no collectives), runs a Tile
kernel via run_bass_kernel_spmd, and re-assembles on the host.

Device-side design (per core, 8 graphs, 16384 nodes):
 - per-node state is "p-major banded" [128, 128]: tile[p, c] = v[p*128 + c];
   graph g owns partitions [16g, 16g+16).
 - P (bf16) is SBUF-resident ([128, 8, 16, 512], 128KB/partition); P^T
   (bf16, host-pretransposed) streams from DRAM once per step for einsum2.
 - The kernel is software-pipelined at QUARTER granularity (2 graphs,
   partitions [32Q, 32Q+32)) so the P^T stream is consumed continuously:
   slot(t, Q) emits [deferred tail of quarter Q-2: d-chain + einsum1],
   [einsum2 + line search of step t-1, quarter Q], [xs row + MLP of step
   t, quarter Q].  The sync DMA queue carries, in priority order: small
   constants, P by-graph, then the P^T chunk ring (6 buffers).
 - Step 7's einsum1/einsum2/line-search are dead (xs_8 unused) and skipped.
 - All PSUM matvec rows are evacuated with full-tile [128, 512] ops
   (garbage partitions never read), then re-banded by scatter DMAs on the
   scalar/gpsimd queues.
"""

import numpy as np
import ml_dtypes

import bass_rust
import concourse.bass as bass
import concourse.tile as tile
from concourse import mybir
from concourse.bass_utils import run_bass_kernel_spmd
from concourse.masks import make_identity

F32 = mybir.dt.float32
BF16 = mybir.dt.bfloat16
BF = ml_dtypes.bfloat16

B = 64          # graphs
NMAX = 2048     # nodes per graph (equal-size, sorted vals_batch)
F = 512         # projection basis dim
HID = 128
NFEAT = 64
NUM_STEPS = 8
STEP_ALPHA = 5.0
NCORES = 8
GPC = B // NCORES            # graphs per core = 8
NPC = GPC * NMAX             # nodes per core = 16384
NCH = NMAX // 128            # n-chunks per graph = 16
FCH = F // 128               # f-chunks = 4
NODE_CH = NPC // 512         # mlp node chunks of 512 = 32
NQ = 4                       # pipeline quarters (2 graphs each)

AX = mybir.AxisListType
OP = mybir.AluOpType
ACT = mybir.ActivationFunctionType

_COMPILED = {}


def _split_sync_waits(nc, maxw=1):
    """Walrus in this container accepts at most one sync wait per
    instruction; split extra waits into preceding engine-local NoOps."""
    ctr = 0
    for f in nc.m.functions:
        for bb in f.blocks:
            insts = bb.instructions
            out = []
            changed = False
            for ins in insts:
                si = ins.sync_info
                waits = list(si.on_wait) if si is not None else []
                if len(waits) > maxw:
                    reg_waits = [w for w in waits if w.wait_reg is not None]
                    imm_waits = [w for w in waits if w.wait_reg is None]
                    nkeep = max(0, maxw - len(reg_waits))
                    keep = imm_waits[:nkeep]
                    extra = imm_waits[nkeep:]
                    for i in range(0, len(extra), maxw):
                        ctr += 1
                        nop = mybir.InstNoOp(name=f"wsplit-{ctr}", ins=[], outs=[])
                        nop.engine = ins.engine
                        nop.sync_info = bass_rust.SyncInfo(
                            on_wait=extra[i : i + maxw], on_update=[]
                        )
                        out.append(nop)
                    ins.sync_info = bass_rust.SyncInfo(
                        on_wait=reg_waits + keep, on_update=list(si.on_update)
                    )
                    changed = True
                out.append(ins)
            if changed:
                bb.instructions = out
    return ctr


def _tau_schedule():
    taus = []
    tau = 0.01
    for _ in range(NUM_STEPS):
        taus.append(tau)
        tau = max(tau * 0.5, 1e-5)
    return taus


def build_nc(debug=False, num_steps=NUM_STEPS, skip=(), pt_bufs=6):
    nc = bass.Bass()

    # ---------------- I/O ----------------
    P_d = nc.declare_dram_parameter("P", [128, GPC, NCH, F], BF16, isOutput=False)
    PT_d = nc.declare_dram_parameter("PT", [GPC, FCH, 128, NMAX], BF16, isOutput=False)
    nfT_d = nc.declare_dram_parameter("nfT", [NFEAT, NPC], BF16, isOutput=False)
    xs0_d = nc.declare_dram_parameter("xs0", [128, 128], F32, isOutput=False)
    xsol_d = nc.declare_dram_parameter("xsol", [128, 128], F32, isOutput=False)
    w1_d = nc.declare_dram_parameter("w1", [NFEAT + 1, HID], BF16, isOutput=False)
    b1_d = nc.declare_dram_parameter("b1", [HID, 1], F32, isOutput=False)
    w2_d = nc.declare_dram_parameter("w2", [HID, 1], BF16, isOutput=False)
    b2_d = nc.declare_dram_parameter("b2", [1, 1], F32, isOutput=False)
    seg_d = nc.declare_dram_parameter("seg", [128, 128], F32, isOutput=False)
    seg2_d = nc.declare_dram_parameter("seg2", [2, 32], F32, isOutput=False)

    preds_o = nc.declare_dram_parameter("preds", [NUM_STEPS, NPC], F32, isOutput=True)
    labels_o = nc.declare_dram_parameter("labels", [NUM_STEPS, NPC], F32, isOutput=True)

    taus = _tau_schedule()
    last = num_steps - 1  # step whose einsum/line-search tail is dead

    with tile.TileContext(nc) as tc:
        with (
            tc.tile_pool(name="res", bufs=1) as res,
            tc.tile_pool(name="ptp", bufs=pt_bufs) as ptp,
            tc.tile_pool(name="hp", bufs=3) as hp,
            tc.tile_pool(name="rows", bufs=4) as rows,
            tc.tile_pool(name="smt", bufs=1) as smt,
            tc.tile_pool(name="mlp_h_ps", bufs=2, space="PSUM") as mlp_h_ps,
            tc.tile_pool(name="mlp_p_ps", bufs=1, space="PSUM") as mlp_p_ps,
            tc.tile_pool(name="ei_ps", bufs=1, space="PSUM") as ei_ps,
            tc.tile_pool(name="ms_ps", bufs=1, space="PSUM") as ms_ps,
        ):
            # ---------------- constants / residents ----------------
            identf = res.tile([128, 128], F32, tag="identf")
            make_identity(nc, identf)
            identb = res.tile([128, 128], BF16, tag="identb")
            make_identity(nc, identb)
            onesf = res.tile([128, 1], F32, tag="onesf")
            nc.vector.memset(onesf, 1.0)

            # Sync queue, priority order: smalls -> P by graph -> PT ring.
            w1 = res.tile([NFEAT + 1, HID], BF16, tag="w1")
            nc.sync.dma_start(out=w1, in_=w1_d[:])
            b1c = res.tile([HID, 1], F32, tag="b1c")
            nc.sync.dma_start(out=b1c, in_=b1_d[:])
            w2 = res.tile([HID, 1], BF16, tag="w2")
            nc.sync.dma_start(out=w2, in_=w2_d[:])
            b2c = res.tile([128, 1], F32, tag="b2c")
            nc.sync.dma_start(
                out=b2c,
                in_=bass.AP(tensor=b2_d, offset=0, ap=[[0, 128], [1, 1]]),
            )
            seg = res.tile([128, 128], F32, tag="seg")
            nc.sync.dma_start(out=seg, in_=seg_d[:])
            seg2 = res.tile([2, 32], F32, tag="seg2")
            nc.sync.dma_start(out=seg2, in_=seg2_d[:])
            xs = res.tile([128, 128], F32, tag="xs")
            nc.sync.dma_start(out=xs, in_=xs0_d[:])
            xsol = res.tile([128, 128], F32, tag="xsol")
            nc.sync.dma_start(out=xsol, in_=xsol_d[:])
            rhsx = res.tile([NFEAT + 1, NPC], BF16, tag="rhsx")
            nc.sync.dma_start(out=rhsx[0:NFEAT, :], in_=nfT_d[:])

            sbP = res.tile([128, GPC, NCH, F], BF16, tag="sbP")
            for g in range(GPC):
                nc.sync.dma_start(out=sbP[:, g], in_=P_d[:, g])

            # ---------------- per-quarter phase emitters ----------------
            def emit_mlp(s, Q):
                # xs row (bf16) into rhsx[64] for this quarter's nodes
                xs_bf = smt.tile([128, 128], BF16, tag="xs_bf", bufs=2)
                nc.vector.tensor_copy(
                    xs_bf[32 * Q : 32 * Q + 32, :], xs[32 * Q : 32 * Q + 32, :]
                )
                nc.scalar.dma_start(
                    out=rhsx[NFEAT : NFEAT + 1, 4096 * Q : 4096 * Q + 4096].rearrange(
                        "o (p c) -> o p c", p=32
                    ),
                    in_=xs_bf[32 * Q : 32 * Q + 32, :],
                )

                pred = smt.tile([128, 128], BF16, tag="pred", bufs=3)
                prow = rows.tile([128, 1024], BF16, tag="prow", bufs=2)
                for g2 in range(2):
                    pp = mlp_p_ps.tile([128, 512], F32, tag="predp")
                    for i in range(4):
                        r = 4 * g2 + i
                        j = 8 * Q + r
                        hpsum = mlp_h_ps.tile([128, 512], F32, tag="hpsum")
                        nc.tensor.matmul(
                            hpsum[:, 0:512],
                            w1,
                            rhsx[:, 512 * j : 512 * j + 512],
                            start=True,
                            stop=True,
                        )
                        hpos = hp.tile([128, 512], BF16, tag="hpos")
                        if r % 2 == 0:
                            nc.vector.tensor_scalar(
                                out=hpos, in0=hpsum,
                                scalar1=b1c, scalar2=0.0,
                                op0=OP.add, op1=OP.max,
                            )
                        else:
                            nc.scalar.activation(
                                out=hpos, in_=hpsum, func=ACT.Relu, bias=b1c,
                            )
                        nc.tensor.matmul(
                            pp[32 * i : 32 * i + 1, :],
                            w2,
                            hpos,
                            start=True,
                            stop=True,
                            tile_position=(0, 32 * i),
                        )
                    # one full-tile evacuation (+b2) of the 4 pred rows
                    dst = prow[:, 512 * g2 : 512 * g2 + 512]
                    if g2 == 0:
                        nc.vector.tensor_scalar(
                            out=dst, in0=pp[:, 0:512],
                            scalar1=b2c, scalar2=None, op0=OP.add,
                        )
                    else:
                        nc.scalar.activation(
                            out=dst, in_=pp[:, 0:512], func=ACT.Identity, bias=b2c,
                        )
                # re-band: chunk r=4*g2+q sits at prow[32q, 512*g2:...]
                for g2 in range(2):
                    for q in range(4):
                        base = 32 * Q + 16 * g2 + 4 * q
                        eng = (nc.scalar, nc.gpsimd)[(2 * g2 + q) % 2]
                        eng.dma_start(
                            out=pred[base : base + 4, :],
                            in_=prow[
                                32 * q : 32 * q + 1, 512 * g2 : 512 * g2 + 512
                            ].rearrange("o (a c) -> o a c", a=4),
                        )
                return pred

            def emit_tail(s, Q, pred):
                # labels = l1norm(xsol - xs_s) for this quarter
                pr = slice(32 * Q, 32 * Q + 32)
                diff = smt.tile([128, 128], F32, tag="diff")
                nc.vector.tensor_sub(diff[pr, :], xsol[pr, :], xs[pr, :])
                lab_part = smt.tile([128, 1], F32, tag="lab_part")
                junk = smt.tile([128, 128], F32, tag="junk", bufs=1)
                nc.scalar.activation(
                    out=junk[pr, :], in_=diff[pr, :], func=ACT.Abs,
                    accum_out=lab_part[pr, :],
                )
                ls_ps = ms_ps.tile([128, 1], F32, tag="ms")
                nc.tensor.matmul(
                    ls_ps[pr, :],
                    seg[pr, pr],
                    lab_part[pr, :],
                    start=True,
                    stop=True,
                    tile_position=(32 * Q, 32 * Q),
                )
                lscale = smt.tile([128, 1], F32, tag="lscale")
                nc.vector.tensor_scalar_max(lscale[pr, :], ls_ps[pr, :], 1e-8)
                nc.vector.reciprocal(lscale[pr, :], lscale[pr, :])
                label = smt.tile([128, 128], F32, tag="label")
                nc.vector.tensor_scalar(
                    out=label[pr, :], in0=diff[pr, :], scalar1=lscale[pr, :],
                    scalar2=None, op0=OP.mult,
                )
                nc.gpsimd.dma_start(
                    out=labels_o[s].rearrange("(p c) -> p c", p=128)[pr], in_=label[pr, :]
                )
                nc.gpsimd.dma_start(
                    out=preds_o[s].rearrange("(p c) -> p c", p=128)[pr], in_=pred[pr, :]
                )
                if s >= last:
                    return  # direction/einsum tail is dead for the final step
                if "tail1" in skip:
                    nc.vector.memset(df_cols[:, 2 * Q : 2 * Q + 2, :], 0.01)
                    return

                # l1norm(pred) scale
                psum_part = smt.tile([128, 1], F32, tag="psum_part")
                junk2 = smt.tile([128, 128], F32, tag="junk", bufs=1)
                nc.scalar.activation(
                    out=junk2[pr, :], in_=pred[pr, :], func=ACT.Abs,
                    accum_out=psum_part[pr, :],
                )
                gs_ps = ms_ps.tile([128, 1], F32, tag="ms")
                nc.tensor.matmul(
                    gs_ps[pr, :],
                    seg[pr, pr],
                    psum_part[pr, :],
                    start=True,
                    stop=True,
                    tile_position=(32 * Q, 32 * Q),
                )
                pscale = smt.tile([128, 1], F32, tag="pscale")
                nc.vector.tensor_scalar_max(pscale[pr, :], gs_ps[pr, :], 1e-8)
                nc.vector.reciprocal(pscale[pr, :], pscale[pr, :])

                # direction d = pred*pscale + 3tau/(xs+tau)
                tau = taus[s]
                pnorm = smt.tile([128, 128], F32, tag="pnorm")
                nc.vector.tensor_scalar(
                    out=pnorm[pr, :], in0=pred[pr, :], scalar1=pscale[pr, :],
                    scalar2=None, op0=OP.mult,
                )
                recv = smt.tile([128, 128], F32, tag="recv", bufs=1)
                nc.vector.tensor_scalar_add(recv[pr, :], xs[pr, :], float(tau))
                nc.vector.reciprocal(recv[pr, :], recv[pr, :])
                d_pm = smt.tile([128, 128], F32, tag="d_pm")
                nc.vector.scalar_tensor_tensor(
                    out=d_pm[pr, :], in0=recv[pr, :], scalar=float(3.0 * tau),
                    in1=pnorm[pr, :], op0=OP.mult, op1=OP.add,
                )
                d_c = smt.tile([128, 32], BF16, tag="d_c", bufs=2)
                if "dct" in skip:
                    nc.vector.memset(d_c, 0.01)
                else:
                    d_bf = smt.tile([128, 128], BF16, tag="d_bf")
                    nc.vector.tensor_copy(d_bf[pr, :], d_pm[pr, :])
                    dct_ps = ms_ps.tile([128, 32], BF16, tag="ms")
                    nc.tensor.transpose(
                        dct_ps, d_bf[pr, :], identb[pr, pr], tile_position=(32 * Q, 0)
                    )
                    nc.vector.tensor_copy(d_c, dct_ps)

                # einsum1: df[g,f] = sum_n P[g,n,f] d[g,n]; 4 streams
                # (gl, khalf) at col 32*(2gl+kh), split-k partials
                dfrow = rows.tile([128, F], F32, tag="dfrow", bufs=1)
                if "e1mm" in skip:
                    nc.vector.memset(dfrow, 0.01)
                else:
                    dfp = ei_ps.tile([128, F], F32, tag="yp0")
                    for k in range(NCH // 2):
                        for gl in range(2):
                            for kh in range(2):
                                c = 2 * gl + kh
                                nc.tensor.matmul(
                                    dfp[32 * c : 32 * c + 1, 0:F],
                                    d_c[
                                        :, 16 * gl + 8 * kh + k : 16 * gl + 8 * kh + k + 1
                                    ],
                                    sbP[:, 2 * Q + gl, 8 * kh + k, 0:F],
                                    start=(k == 0),
                                    stop=(k == NCH // 2 - 1),
                                    tile_position=(0, 32 * c),
                                )
                    if Q % 2 == 0:
                        nc.scalar.copy(dfrow, dfp[:, 0:F])
                    else:
                        nc.vector.tensor_copy(dfrow, dfp[:, 0:F])
                if "dfc" in skip:
                    nc.vector.memset(df_cols[:, 2 * Q : 2 * Q + 2, :], 0.01)
                    return
                # per (gl, kh) row: 4 transposes -> [128, FCH] psum columns;
                # kh partials then summed into df_cols (one PSUM input max)
                dfc_sb = smt.tile([128, FCH], F32, tag="dfc_sb", bufs=2)
                for gl in range(2):
                    for kh in range(2):
                        q = 2 * gl + kh
                        dfc_ps = ms_ps.tile([128, FCH], F32, tag="ms")
                        for fc in range(FCH):
                            nc.tensor.transpose(
                                dfc_ps[:, fc : fc + 1],
                                dfrow[32 * q : 32 * q + 1, 128 * fc : 128 * fc + 128],
                                onesf[32 * q : 32 * q + 1, 0:1],
                                tile_position=(32 * q, 0),
                            )
                        if kh == 0:
                            nc.vector.tensor_copy(dfc_sb, dfc_ps)
                        else:
                            nc.vector.tensor_tensor(
                                out=df_cols[:, 2 * Q + gl, :],
                                in0=dfc_sb,
                                in1=dfc_ps,
                                op=OP.add,
                            )

            def emit_e2ls(s, Q):
                # einsum2: y[g,n] = sum_f PT[g,f,n] df[g,f]; 4 streams
                # (gl, fhalf) at col 32*(2gl+fh), split-f partials
                pr = slice(32 * Q, 32 * Q + 32)
                if "e2" in skip:
                    y_pm0 = smt.tile([128, 128], F32, tag="y_pm")
                    nc.vector.memset(y_pm0[pr, :], 0.05)
                    ay0 = smt.tile([128, 128], F32, tag="ay", bufs=1)
                    nc.vector.tensor_scalar(
                        out=ay0[pr, :], in0=y_pm0[pr, :], scalar1=0.05,
                        scalar2=None, op0=OP.mult,
                    )
                    nc.vector.tensor_add(xs[pr, :], xs[pr, :], ay0[pr, :])
                    return
                yps = [
                    ei_ps.tile([128, 512], F32, tag=f"yp{j}", name=f"yp{j}")
                    for j in range(4)
                ]
                yrow = rows.tile([128, 2048], BF16, tag="yrow", bufs=1)
                for fc2 in range(2):
                    ptts = {}
                    for gl in range(2):
                        for fh in range(2):
                            ptt = ptp.tile([128, NMAX], BF16, tag="ptt", name="ptt")
                            nc.sync.dma_start(
                                out=ptt, in_=PT_d[2 * Q + gl, 2 * fh + fc2]
                            )
                            ptts[(gl, fh)] = ptt
                    for j in range(4):
                        for gl in range(2):
                            for fh in range(2):
                                c = 2 * gl + fh
                                nc.tensor.matmul(
                                    yps[j][32 * c : 32 * c + 1, 0:512],
                                    df_cols[
                                        :, 2 * Q + gl, 2 * fh + fc2 : 2 * fh + fc2 + 1
                                    ],
                                    ptts[(gl, fh)][:, 512 * j : 512 * j + 512],
                                    start=(fc2 == 0),
                                    stop=(fc2 == 1),
                                    tile_position=(0, 32 * c),
                                )
                for j in range(4):
                    dst = yrow[:, 512 * j : 512 * j + 512]
                    if j % 2 == 0:
                        nc.vector.tensor_copy(dst, yps[j][:, 0:512])
                    else:
                        nc.scalar.copy(dst, yps[j][:, 0:512])
                for gl in range(2):
                    for fh in range(2):
                        c = 2 * gl + fh
                        dst = (y_pmA, y_pmB)[fh]
                        nc.gpsimd.dma_start(
                            out=dst[16 * (2 * Q + gl) : 16 * (2 * Q + gl) + 16, :],
                            in_=yrow[32 * c : 32 * c + 1, :].rearrange(
                                "o (p c) -> o p c", p=16
                            ),
                        )
                y_pm = smt.tile([128, 128], F32, tag="y_pm")
                nc.vector.tensor_add(y_pm[pr, :], y_pmA[pr, :], y_pmB[pr, :])

                if "ls" in skip:
                    ay0 = smt.tile([128, 128], F32, tag="ay", bufs=1)
                    nc.vector.tensor_scalar(
                        out=ay0[pr, :], in0=y_pm[pr, :], scalar1=0.05,
                        scalar2=None, op0=OP.mult,
                    )
                    nc.vector.tensor_add(xs[pr, :], xs[pr, :], ay0[pr, :])
                    return
                # line search + state update
                q = smt.tile([128, 128], F32, tag="q", bufs=1)
                nc.vector.tensor_scalar(
                    out=q[pr, :], in0=y_pm[pr, :], scalar1=-1.0, scalar2=1e-30,
                    op0=OP.mult, op1=OP.max,
                )
                nc.vector.reciprocal(q[pr, :], q[pr, :])
                stp = smt.tile([128, 128], F32, tag="stp", bufs=1)
                nc.vector.tensor_mul(stp[pr, :], xs[pr, :], q[pr, :])
                smin = smt.tile([128, 1], F32, tag="smin")
                nc.vector.tensor_reduce(
                    out=smin[pr, :], in_=stp[pr, :], axis=AX.X, op=OP.min
                )
                smin_ps = ms_ps.tile([128, 32], F32, tag="ms")
                nc.tensor.transpose(
                    smin_ps[0:1, :], smin[pr, :], identf[pr, pr],
                    tile_position=(32 * Q, 0),
                )
                smin_row = smt.tile([1, 32], F32, tag="smin_row")
                nc.scalar.copy(smin_row, smin_ps[0:1, :])
                amin_row = smt.tile([1, 2], F32, tag="amin_row")
                nc.vector.tensor_reduce(
                    out=amin_row,
                    in_=smin_row.rearrange("o (g b) -> o g b", g=2),
                    axis=AX.X,
                    op=OP.min,
                )
                nc.vector.tensor_scalar(
                    out=amin_row, in0=amin_row, scalar1=float(STEP_ALPHA),
                    scalar2=0.995, op0=OP.min, op1=OP.mult,
                )
                a8_ps = ms_ps.tile([2, 1], F32, tag="ms")
                nc.tensor.transpose(a8_ps, amin_row, identf[0:1, 0:1])
                a8 = smt.tile([2, 1], F32, tag="a8")
                nc.scalar.copy(a8, a8_ps)
                acol_ps = ms_ps.tile([128, 1], F32, tag="ms")
                nc.tensor.matmul(
                    acol_ps[pr, :], seg2, a8, start=True, stop=True,
                    tile_position=(0, 32 * Q),
                )
                acol = smt.tile([128, 1], F32, tag="acol")
                nc.vector.tensor_copy(acol[pr, :], acol_ps[pr, :])

                ay = smt.tile([128, 128], F32, tag="ay", bufs=1)
                nc.vector.tensor_scalar(
                    out=ay[pr, :], in0=y_pm[pr, :], scalar1=acol[pr, :],
                    scalar2=None, op0=OP.mult,
                )
                nc.vector.tensor_add(xs[pr, :], xs[pr, :], ay[pr, :])

            # ---------------- pipelined slot loop ----------------
            df_cols = smt.tile([128, GPC, FCH], BF16, tag="df_cols")
            y_pmA = smt.tile([128, 128], BF16, tag="y_pmA")
            y_pmB = smt.tile([128, 128], BF16, tag="y_pmB")
            preds_live = {}  # (s, Q) -> pred tile awaiting its tail

            for t in range(num_steps + 1):
                for Q in range(NQ):
                    tq = Q - 2
                    ts_ = t
                    if tq < 0:
                        tq += NQ
                        ts_ = t - 1
                    if 0 <= ts_ <= num_steps - 1 and (ts_, tq) in preds_live:
                        emit_tail(ts_, tq, preds_live.pop((ts_, tq)))
                    if 1 <= t <= num_steps - 1:
                        emit_e2ls(t - 1, Q)
                    if t <= num_steps - 1:
                        preds_live[(t, Q)] = emit_mlp(t, Q)

    _split_sync_waits(nc, maxw=1)
    return nc


def _seg_mats():
    seg = np.zeros((128, 128), np.float32)
    for g in range(GPC):
        seg[16 * g : 16 * g + 16, 16 * g : 16 * g + 16] = 1.0
    seg2 = np.zeros((2, 32), np.float32)
    for gl in range(2):
        seg2[gl, 16 * gl : 16 * gl + 16] = 1.0
    return seg, seg2


def _prep_core_inputs(core, proj, x_start, x_solution, node_feat, W1, b1, W2, b2):
    g0 = core * GPC
    n0 = core * NPC
    Pc = proj[g0 : g0 + GPC]  # [8, 2048, 512] f32
    P_bf = np.ascontiguousarray(
        Pc.reshape(GPC, NCH, 128, F).transpose(2, 0, 1, 3)
    ).astype(BF)
    PT_bf = np.ascontiguousarray(Pc.transpose(0, 2, 1)).reshape(
        GPC, FCH, 128, NMAX
    ).astype(BF)
    nfT = np.ascontiguousarray(node_feat[n0 : n0 + NPC].T).astype(BF)
    seg, seg2 = _seg_mats()
    return {
        "P": P_bf,
        "PT": PT_bf,
        "nfT": nfT,
        "xs0": x_start[n0 : n0 + NPC].reshape(128, 128).astype(np.float32),
        "xsol": x_solution[n0 : n0 + NPC].reshape(128, 128).astype(np.float32),
        "w1": W1.astype(BF),
        "b1": b1.reshape(HID, 1).astype(np.float32),
        "w2": W2.reshape(HID, 1).astype(BF),
        "b2": b2.reshape(1, 1).astype(np.float32),
        "seg": seg,
        "seg2": seg2,
    }


def _numpy_fallback(x_start, x_solution, node_feat, proj_matrix, W1, b1, W2, b2, batch):
    """General (ragged) reference implementation in numpy, used only if
    vals_batch is not the expected equal-size pattern."""
    nb = proj_matrix.shape[0]
    batch = batch.astype(np.int64)
    counts = np.bincount(batch, minlength=nb)
    offsets = np.cumsum(counts) - counts
    pos = np.arange(batch.shape[0]) - offsets[batch]

    def l1norm(x):
        s = np.zeros(nb, x.dtype)
        np.add.at(s, batch, np.abs(x))
        return x / np.clip(s, 1e-8, None)[batch]

    def to_dense(x):
        dense = np.zeros((nb, NMAX), x.dtype)
        m = pos < NMAX
        dense[batch[m], pos[m]] = x[m]
        return dense

    def line_search(x, dvec):
        neg = dvec < 0
        step = np.where(neg, x / np.where(neg, -dvec, 1.0), STEP_ALPHA)
        a = np.full(nb, np.inf, step.dtype)
        np.minimum.at(a, batch, step)
        return np.minimum(a, STEP_ALPHA)[batch]

    def gnn(x):
        h = np.concatenate([node_feat, x[:, None]], axis=-1)
        h = np.maximum(h @ W1 + b1, 0.0)
        return (h @ W2 + b2)[:, 0]

    tau = 0.01
    xs = x_start.astype(np.float32)
    preds, labels = [], []
    for _ in range(NUM_STEPS):
        pred = gnn(xs)
        preds.append(pred)
        labels.append(l1norm(x_solution - xs))
        p = l1norm(pred)
        direction = p + 3.0 * tau / (xs + tau)
        tau = max(tau * 0.5, 1e-5)
        d_dense = to_dense(direction)
        df = np.einsum("bnf,bn->bf", proj_matrix, d_dense)
        proj_dense = np.einsum("bnf,bf->bn", proj_matrix, df)
        proj_flat = proj_dense[batch, np.minimum(pos, NMAX - 1)]
        proj_flat = np.where(pos < NMAX, proj_flat, 0.0)
        alpha = line_search(xs, proj_flat) * 0.995
        xs = xs + alpha * proj_flat
    return np.stack(preds, 1).astype(np.float32), np.stack(labels, 1).astype(np.float32)


def run_on_hw(inputs_list, debug=False):
    key = "plain"
    if key not in _COMPILED:
        _COMPILED[key] = build_nc()
    nc = _COMPILED[key]
    return run_bass_kernel_spmd(nc, inputs_list, list(range(NCORES))).results


def kernel(x_start, x_solution, node_feat, proj_matrix, W1, b1, W2, b2, vals_batch):
    expected = np.repeat(np.arange(B, dtype=np.int64), NMAX)
    vb = np.asarray(vals_batch)
    if vb.shape != expected.shape or not np.array_equal(
        vb.astype(np.int64), expected
    ):
        return _numpy_fallback(
            np.asarray(x_start, np.float32),
            np.asarray(x_solution, np.float32),
            np.asarray(node_feat, np.float32),
            np.asarray(proj_matrix, np.float32),
            np.asarray(W1, np.float32),
            np.asarray(b1, np.float32),
            np.asarray(W2, np.float32),
            np.asarray(b2, np.float32),
            vb,
        )

    x_start = np.asarray(x_start, np.float32)
    x_solution = np.asarray(x_solution, np.float32)
    node_feat = np.asarray(node_feat, np.float32)
    proj_matrix = np.asarray(proj_matrix, np.float32)
    W1 = np.asarray(W1, np.float32)
    b1 = np.asarray(b1, np.float32)
    W2 = np.asarray(W2, np.float32)
    b2 = np.asarray(b2, np.float32)

    ins = [
        _prep_core_inputs(c, proj_matrix, x_start, x_solution, node_feat, W1, b1, W2, b2)
        for c in range(NCORES)
    ]
    results = run_on_hw(ins)
    preds = np.concatenate(
        [results[c]["preds"].T for c in range(NCORES)], axis=0
    ).astype(np.float32)
    labels = np.concatenate(
        [results[c]["labels"].T for c in range(NCORES)], axis=0
    ).astype(np.float32)
    return preds, labels


# revision 27
# speedup vs baseline: 1.1698x; 1.1698x over previous
"""Trainium2 Bass kernel for nn_CycleGNN (8-step projected-direction solver).

Contract: kernel(**inputs) takes the FULL unsharded numpy inputs (keyed as in
setup_inputs()) and returns the full output (preds, labels), each
[131072, 8] float32.  Internally shards the 64 graphs across 8 NeuronCores
(8 graphs per core, graphs never interact -> no collectives), runs a Tile
kernel via run_bass_kernel_spmd, and re-assembles on the host.

Device-side design (per core, 8 graphs, 16384 nodes):
 - per-node state is "p-major banded" [128, 128]: tile[p, c] = v[p*128 + c];
   graph g owns partitions [16g, 16g+16).
 - P (bf16) is SBUF-resident ([128, 8, 16, 512], 128KB/partition); P^T
   (bf16, host-pretransposed) streams from DRAM once per step for einsum2.
 - The kernel is software-pipelined at QUARTER granularity (2 graphs,
   partitions [32Q, 32Q+32)) so the P^T stream is consumed continuously:
   slot(t, Q) emits [deferred tail of quarter Q-2: d-chain + einsum1],
   [einsum2 + line search of step t-1, quarter Q], [xs row + MLP of step
   t, quarter Q].  The sync DMA queue carries, in priority order: small
   constants, P by-graph, then the P^T chunk ring (6 buffers).
 - Step 7's einsum1/einsum2/line-search are dead (xs_8 unused) and skipped.
 - All PSUM matvec rows are evacuated with full-tile [128, 512] ops
   (garbage partitions never read), then re-banded by scatter DMAs on the
   scalar/gpsimd queues.
"""

import numpy as np
import ml_dtypes

import bass_rust
import concourse.bass as bass
import concourse.tile as tile
from concourse import mybir
from concourse.bass_utils import run_bass_kernel_spmd
from concourse.masks import make_identity

F32 = mybir.dt.float32
BF16 = mybir.dt.bfloat16
BF = ml_dtypes.bfloat16

B = 64          # graphs
NMAX = 2048     # nodes per graph (equal-size, sorted vals_batch)
F = 512         # projection basis dim
HID = 128
NFEAT = 64
NUM_STEPS = 8
STEP_ALPHA = 5.0
NCORES = 8
GPC = B // NCORES            # graphs per core = 8
NPC = GPC * NMAX             # nodes per core = 16384
NCH = NMAX // 128            # n-chunks per graph = 16
FCH = F // 128               # f-chunks = 4
NODE_CH = NPC // 512         # mlp node chunks of 512 = 32
NQ = 4                       # pipeline quarters (2 graphs each)

AX = mybir.AxisListType
OP = mybir.AluOpType
ACT = mybir.ActivationFunctionType

_COMPILED = {}


def _split_sync_waits(nc, maxw=1):
    """Walrus in this container accepts at most one sync wait per
    instruction; split extra waits into preceding engine-local NoOps."""
    ctr = 0
    for f in nc.m.functions:
        for bb in f.blocks:
            insts = bb.instructions
            out = []
            changed = False
            for ins in insts:
                si = ins.sync_info
                waits = list(si.on_wait) if si is not None else []
                if len(waits) > maxw:
                    reg_waits = [w for w in waits if w.wait_reg is not None]
                    imm_waits = [w for w in waits if w.wait_reg is None]
                    nkeep = max(0, maxw - len(reg_waits))
                    keep = imm_waits[:nkeep]
                    extra = imm_waits[nkeep:]
                    for i in range(0, len(extra), maxw):
                        ctr += 1
                        nop = mybir.InstNoOp(name=f"wsplit-{ctr}", ins=[], outs=[])
                        nop.engine = ins.engine
                        nop.sync_info = bass_rust.SyncInfo(
                            on_wait=extra[i : i + maxw], on_update=[]
                        )
                        out.append(nop)
                    ins.sync_info = bass_rust.SyncInfo(
                        on_wait=reg_waits + keep, on_update=list(si.on_update)
                    )
                    changed = True
                out.append(ins)
            if changed:
                bb.instructions = out
    return ctr


def _tau_schedule():
    taus = []
    tau = 0.01
    for _ in range(NUM_STEPS):
        taus.append(tau)
        tau = max(tau * 0.5, 1e-5)
    return taus


def build_nc(debug=False, num_steps=NUM_STEPS, skip=(), pt_bufs=6):
    nc = bass.Bass()

    # ---------------- I/O ----------------
    P_d = nc.declare_dram_parameter("P", [128, GPC, NCH, F], BF16, isOutput=False)
    PT_d = nc.declare_dram_parameter("PT", [GPC, FCH, 128, NMAX], BF16, isOutput=False)
    nfT_d = nc.declare_dram_parameter("nfT", [NFEAT, NPC], BF16, isOutput=False)
    xs0_d = nc.declare_dram_parameter("xs0", [128, 128], F32, isOutput=False)
    xsol_d = nc.declare_dram_parameter("xsol", [128, 128], F32, isOutput=False)
    w1_d = nc.declare_dram_parameter("w1", [NFEAT + 1, HID], BF16, isOutput=False)
    b1_d = nc.declare_dram_parameter("b1", [HID, 1], F32, isOutput=False)
    w2_d = nc.declare_dram_parameter("w2", [HID, 1], BF16, isOutput=False)
    b2_d = nc.declare_dram_parameter("b2", [1, 1], F32, isOutput=False)
    seg_d = nc.declare_dram_parameter("seg", [128, 128], F32, isOutput=False)
    seg2_d = nc.declare_dram_parameter("seg2", [2, 32], F32, isOutput=False)

    preds_o = nc.declare_dram_parameter("preds", [NUM_STEPS, NPC], F32, isOutput=True)
    labels_o = nc.declare_dram_parameter("labels", [NUM_STEPS, NPC], F32, isOutput=True)

    taus = _tau_schedule()
    last = num_steps - 1  # step whose einsum/line-search tail is dead

    with tile.TileContext(nc) as tc:
        with (
            tc.tile_pool(name="res", bufs=1) as res,
            tc.tile_pool(name="ptp", bufs=pt_bufs) as ptp,
            tc.tile_pool(name="hp", bufs=3) as hp,
            tc.tile_pool(name="rows", bufs=4) as rows,
            tc.tile_pool(name="smt", bufs=1) as smt,
            tc.tile_pool(name="mlp_h_ps", bufs=2, space="PSUM") as mlp_h_ps,
            tc.tile_pool(name="mlp_p_ps", bufs=1, space="PSUM") as mlp_p_ps,
            tc.tile_pool(name="ei_ps", bufs=1, space="PSUM") as ei_ps,
            tc.tile_pool(name="ms_ps", bufs=1, space="PSUM") as ms_ps,
        ):
            # ---------------- constants / residents ----------------
            identf = res.tile([128, 128], F32, tag="identf")
            make_identity(nc, identf)
            identb = res.tile([128, 128], BF16, tag="identb")
            make_identity(nc, identb)
            onesf = res.tile([128, 1], F32, tag="onesf")
            nc.vector.memset(onesf, 1.0)

            # Sync queue, priority order: smalls -> P by graph -> PT ring.
            w1 = res.tile([NFEAT + 1, HID], BF16, tag="w1")
            nc.sync.dma_start(out=w1, in_=w1_d[:])
            b1c = res.tile([HID, 1], F32, tag="b1c")
            nc.sync.dma_start(out=b1c, in_=b1_d[:])
            w2 = res.tile([HID, 1], BF16, tag="w2")
            nc.sync.dma_start(out=w2, in_=w2_d[:])
            b2c = res.tile([128, 1], F32, tag="b2c")
            nc.sync.dma_start(
                out=b2c,
                in_=bass.AP(tensor=b2_d, offset=0, ap=[[0, 128], [1, 1]]),
            )
            seg = res.tile([128, 128], F32, tag="seg")
            nc.sync.dma_start(out=seg, in_=seg_d[:])
            seg2 = res.tile([2, 32], F32, tag="seg2")
            nc.sync.dma_start(out=seg2, in_=seg2_d[:])
            xs = res.tile([128, 128], F32, tag="xs")
            nc.sync.dma_start(out=xs, in_=xs0_d[:])
            xsol = res.tile([128, 128], F32, tag="xsol")
            nc.sync.dma_start(out=xsol, in_=xsol_d[:])
            rhsx = res.tile([NFEAT + 1, NPC], BF16, tag="rhsx")
            nc.sync.dma_start(out=rhsx[0:NFEAT, :], in_=nfT_d[:])

            sbP = res.tile([128, GPC, NCH, F], BF16, tag="sbP")
            for g in range(GPC):
                nc.sync.dma_start(out=sbP[:, g], in_=P_d[:, g])

            # ---------------- per-quarter phase emitters ----------------
            def emit_mlp(s, Q):
                # xs row (bf16) into rhsx[64] for this quarter's nodes
                xs_bf = smt.tile([128, 128], BF16, tag="xs_bf", bufs=2)
                nc.vector.tensor_copy(
                    xs_bf[32 * Q : 32 * Q + 32, :], xs[32 * Q : 32 * Q + 32, :]
                )
                nc.scalar.dma_start(
                    out=rhsx[NFEAT : NFEAT + 1, 4096 * Q : 4096 * Q + 4096].rearrange(
                        "o (p c) -> o p c", p=32
                    ),
                    in_=xs_bf[32 * Q : 32 * Q + 32, :],
                )

                pred = smt.tile([128, 128], BF16, tag="pred", bufs=3)
                prow = rows.tile([128, 1024], BF16, tag="prow", bufs=2)
                for g2 in range(2):
                    pp = mlp_p_ps.tile([128, 512], F32, tag="predp")
                    for i in range(4):
                        r = 4 * g2 + i
                        j = 8 * Q + r
                        hpsum = mlp_h_ps.tile([128, 512], F32, tag="hpsum")
                        nc.tensor.matmul(
                            hpsum[:, 0:512],
                            w1,
                            rhsx[:, 512 * j : 512 * j + 512],
                            start=True,
                            stop=True,
                        )
                        hpos = hp.tile([128, 512], BF16, tag="hpos")
                        if r % 2 == 0:
                            nc.vector.tensor_scalar(
                                out=hpos, in0=hpsum,
                                scalar1=b1c, scalar2=0.0,
                                op0=OP.add, op1=OP.max,
                            )
                        else:
                            nc.scalar.activation(
                                out=hpos, in_=hpsum, func=ACT.Relu, bias=b1c,
                            )
                        nc.tensor.matmul(
                            pp[32 * i : 32 * i + 1, :],
                            w2,
                            hpos,
                            start=True,
                            stop=True,
                            tile_position=(0, 32 * i),
                        )
                    # one full-tile evacuation (+b2) of the 4 pred rows
                    dst = prow[:, 512 * g2 : 512 * g2 + 512]
                    if g2 == 0:
                        nc.vector.tensor_scalar(
                            out=dst, in0=pp[:, 0:512],
                            scalar1=b2c, scalar2=None, op0=OP.add,
                        )
                    else:
                        nc.scalar.activation(
                            out=dst, in_=pp[:, 0:512], func=ACT.Identity, bias=b2c,
                        )
                # re-band: chunk r=4*g2+q sits at prow[32q, 512*g2:...]
                for g2 in range(2):
                    for q in range(4):
                        base = 32 * Q + 16 * g2 + 4 * q
                        eng = (nc.scalar, nc.gpsimd)[(2 * g2 + q) % 2]
                        eng.dma_start(
                            out=pred[base : base + 4, :],
                            in_=prow[
                                32 * q : 32 * q + 1, 512 * g2 : 512 * g2 + 512
                            ].rearrange("o (a c) -> o a c", a=4),
                        )
                return pred

            def emit_tail(s, Q, pred):
                # labels = l1norm(xsol - xs_s) for this quarter
                pr = slice(32 * Q, 32 * Q + 32)
                diff = smt.tile([128, 128], F32, tag="diff")
                nc.vector.tensor_sub(diff[pr, :], xsol[pr, :], xs[pr, :])
                lab_part = smt.tile([128, 1], F32, tag="lab_part")
                junk = smt.tile([128, 128], F32, tag="junk", bufs=1)
                nc.scalar.activation(
                    out=junk[pr, :], in_=diff[pr, :], func=ACT.Abs,
                    accum_out=lab_part[pr, :],
                )
                ls_ps = ms_ps.tile([128, 1], F32, tag="ms")
                nc.tensor.matmul(
                    ls_ps[pr, :],
                    seg[pr, pr],
                    lab_part[pr, :],
                    start=True,
                    stop=True,
                    tile_position=(32 * Q, 32 * Q),
                )
                lscale = smt.tile([128, 1], F32, tag="lscale")
                nc.vector.tensor_scalar_max(lscale[pr, :], ls_ps[pr, :], 1e-8)
                nc.vector.reciprocal(lscale[pr, :], lscale[pr, :])
                label = smt.tile([128, 128], F32, tag="label")
                nc.vector.tensor_scalar(
                    out=label[pr, :], in0=diff[pr, :], scalar1=lscale[pr, :],
                    scalar2=None, op0=OP.mult,
                )
                nc.gpsimd.dma_start(
                    out=labels_o[s].rearrange("(p c) -> p c", p=128)[pr], in_=label[pr, :]
                )
                nc.gpsimd.dma_start(
                    out=preds_o[s].rearrange("(p c) -> p c", p=128)[pr], in_=pred[pr, :]
                )
                if s >= last:
                    return  # direction/einsum tail is dead for the final step
                if "tail1" in skip:
                    nc.vector.memset(df_cols[:, 2 * Q : 2 * Q + 2, :], 0.01)
                    return

                # l1norm(pred) scale
                psum_part = smt.tile([128, 1], F32, tag="psum_part")
                junk2 = smt.tile([128, 128], F32, tag="junk", bufs=1)
                nc.scalar.activation(
                    out=junk2[pr, :], in_=pred[pr, :], func=ACT.Abs,
                    accum_out=psum_part[pr, :],
                )
                gs_ps = ms_ps.tile([128, 1], F32, tag="ms")
                nc.tensor.matmul(
                    gs_ps[pr, :],
                    seg[pr, pr],
                    psum_part[pr, :],
                    start=True,
                    stop=True,
                    tile_position=(32 * Q, 32 * Q),
                )
                pscale = smt.tile([128, 1], F32, tag="pscale")
                nc.vector.tensor_scalar_max(pscale[pr, :], gs_ps[pr, :], 1e-8)
                nc.vector.reciprocal(pscale[pr, :], pscale[pr, :])

                # direction d = pred*pscale + 3tau/(xs+tau)
                tau = taus[s]
                pnorm = smt.tile([128, 128], F32, tag="pnorm")
                nc.vector.tensor_scalar(
                    out=pnorm[pr, :], in0=pred[pr, :], scalar1=pscale[pr, :],
                    scalar2=None, op0=OP.mult,
                )
                recv = smt.tile([128, 128], F32, tag="recv", bufs=1)
                nc.vector.tensor_scalar_add(recv[pr, :], xs[pr, :], float(tau))
                nc.vector.reciprocal(recv[pr, :], recv[pr, :])
                d_pm = smt.tile([128, 128], F32, tag="d_pm")
                nc.vector.scalar_tensor_tensor(
                    out=d_pm[pr, :], in0=recv[pr, :], scalar=float(3.0 * tau),
                    in1=pnorm[pr, :], op0=OP.mult, op1=OP.add,
                )
                d_c = smt.tile([128, 32], BF16, tag="d_c", bufs=2)
                if "dct" in skip:
                    nc.vector.memset(d_c, 0.01)
                else:
                    d_bf = smt.tile([128, 128], BF16, tag="d_bf")
                    nc.vector.tensor_copy(d_bf[pr, :], d_pm[pr, :])
                    dct_ps = ms_ps.tile([128, 32], BF16, tag="ms")
                    nc.tensor.transpose(
                        dct_ps, d_bf[pr, :], identb[pr, pr], tile_position=(32 * Q, 0)
                    )
                    nc.vector.tensor_copy(d_c, dct_ps)

                # einsum1: df[g,f] = sum_n P[g,n,f] d[g,n]; 4 streams
                # (gl, khalf) at col 32*(2gl+kh), split-k partials
                dfrow = rows.tile([128, F], F32, tag="dfrow", bufs=1)
                if "e1mm" in skip:
                    nc.vector.memset(dfrow, 0.01)
                else:
                    dfp = mlp_h_ps.tile([128, F], F32, tag="hpsum")
                    for k in range(NCH // 2):
                        for gl in range(2):
                            for kh in range(2):
                                c = 2 * gl + kh
                                nc.tensor.matmul(
                                    dfp[32 * c : 32 * c + 1, 0:F],
                                    d_c[
                                        :, 16 * gl + 8 * kh + k : 16 * gl + 8 * kh + k + 1
                                    ],
                                    sbP[:, 2 * Q + gl, 8 * kh + k, 0:F],
                                    start=(k == 0),
                                    stop=(k == NCH // 2 - 1),
                                    tile_position=(0, 32 * c),
                                )
                    if Q % 2 == 0:
                        nc.scalar.copy(dfrow, dfp[:, 0:F])
                    else:
                        nc.vector.tensor_copy(dfrow, dfp[:, 0:F])
                if "dfc" in skip:
                    nc.vector.memset(df_cols[:, 2 * Q : 2 * Q + 2, :], 0.01)
                    return
                # per (gl, kh) row: 4 transposes -> [128, FCH] psum columns;
                # kh partials then summed into df_cols (one PSUM input max)
                dfc_sb = smt.tile([128, FCH], F32, tag="dfc_sb", bufs=2)
                for gl in range(2):
                    for kh in range(2):
                        q = 2 * gl + kh
                        dfc_ps = ms_ps.tile([128, FCH], F32, tag="ms")
                        for fc in range(FCH):
                            nc.tensor.transpose(
                                dfc_ps[:, fc : fc + 1],
                                dfrow[32 * q : 32 * q + 1, 128 * fc : 128 * fc + 128],
                                onesf[32 * q : 32 * q + 1, 0:1],
                                tile_position=(32 * q, 0),
                            )
                        if kh == 0:
                            nc.vector.tensor_copy(dfc_sb, dfc_ps)
                        else:
                            nc.vector.tensor_tensor(
                                out=df_cols[:, 2 * Q + gl, :],
                                in0=dfc_sb,
                                in1=dfc_ps,
                                op=OP.add,
                            )

            def emit_e2ls(s, Q):
                # einsum2: y[g,n] = sum_f PT[g,f,n] df[g,f]; 4 streams
                # (gl, fhalf) at col 32*(2gl+fh), split-f partials
                pr = slice(32 * Q, 32 * Q + 32)
                if "e2" in skip:
                    y_pm0 = smt.tile([128, 128], F32, tag="y_pm")
                    nc.vector.memset(y_pm0[pr, :], 0.05)
                    ay0 = smt.tile([128, 128], F32, tag="ay", bufs=1)
                    nc.vector.tensor_scalar(
                        out=ay0[pr, :], in0=y_pm0[pr, :], scalar1=0.05,
                        scalar2=None, op0=OP.mult,
                    )
                    nc.vector.tensor_add(xs[pr, :], xs[pr, :], ay0[pr, :])
                    return
                yps = [
                    ei_ps.tile([128, 512], F32, tag=f"yp{j}", name=f"yp{j}")
                    for j in range(4)
                ]
                yrow = rows.tile([128, 2048], BF16, tag="yrow", bufs=1)
                for fc2 in range(2):
                    ptts = {}
                    for gl in range(2):
                        for fh in range(2):
                            ptt = ptp.tile([128, NMAX], BF16, tag="ptt", name="ptt")
                            nc.sync.dma_start(
                                out=ptt, in_=PT_d[2 * Q + gl, 2 * fh + fc2]
                            )
                            ptts[(gl, fh)] = ptt
                    for j in range(4):
                        for gl in range(2):
                            for fh in range(2):
                                c = 2 * gl + fh
                                nc.tensor.matmul(
                                    yps[j][32 * c : 32 * c + 1, 0:512],
                                    df_cols[
                                        :, 2 * Q + gl, 2 * fh + fc2 : 2 * fh + fc2 + 1
                                    ],
                                    ptts[(gl, fh)][:, 512 * j : 512 * j + 512],
                                    start=(fc2 == 0),
                                    stop=(fc2 == 1),
                                    tile_position=(0, 32 * c),
                                )
                for j in range(4):
                    dst = yrow[:, 512 * j : 512 * j + 512]
                    if j % 2 == 0:
                        nc.vector.tensor_copy(dst, yps[j][:, 0:512])
                    else:
                        nc.scalar.copy(dst, yps[j][:, 0:512])
                for gl in range(2):
                    for fh in range(2):
                        c = 2 * gl + fh
                        dst = (y_pmA, y_pmB)[fh]
                        nc.gpsimd.dma_start(
                            out=dst[16 * (2 * Q + gl) : 16 * (2 * Q + gl) + 16, :],
                            in_=yrow[32 * c : 32 * c + 1, :].rearrange(
                                "o (p c) -> o p c", p=16
                            ),
                        )
                y_pm = smt.tile([128, 128], F32, tag="y_pm")
                nc.vector.tensor_add(y_pm[pr, :], y_pmA[pr, :], y_pmB[pr, :])

                if "ls" in skip:
                    ay0 = smt.tile([128, 128], F32, tag="ay", bufs=1)
                    nc.vector.tensor_scalar(
                        out=ay0[pr, :], in0=y_pm[pr, :], scalar1=0.05,
                        scalar2=None, op0=OP.mult,
                    )
                    nc.vector.tensor_add(xs[pr, :], xs[pr, :], ay0[pr, :])
                    return
                # line search + state update
                q = smt.tile([128, 128], F32, tag="q", bufs=1)
                nc.vector.tensor_scalar(
                    out=q[pr, :], in0=y_pm[pr, :], scalar1=-1.0, scalar2=1e-30,
                    op0=OP.mult, op1=OP.max,
                )
                nc.vector.reciprocal(q[pr, :], q[pr, :])
                stp = smt.tile([128, 128], F32, tag="stp", bufs=1)
                nc.vector.tensor_mul(stp[pr, :], xs[pr, :], q[pr, :])
                smin = smt.tile([128, 1], F32, tag="smin")
                nc.vector.tensor_reduce(
                    out=smin[pr, :], in_=stp[pr, :], axis=AX.X, op=OP.min
                )
                smin_ps = ms_ps.tile([128, 32], F32, tag="ms")
                nc.tensor.transpose(
                    smin_ps[0:1, :], smin[pr, :], identf[pr, pr],
                    tile_position=(32 * Q, 0),
                )
                smin_row = smt.tile([1, 32], F32, tag="smin_row")
                nc.scalar.copy(smin_row, smin_ps[0:1, :])
                amin_row = smt.tile([1, 2], F32, tag="amin_row")
                nc.vector.tensor_reduce(
                    out=amin_row,
                    in_=smin_row.rearrange("o (g b) -> o g b", g=2),
                    axis=AX.X,
                    op=OP.min,
                )
                nc.vector.tensor_scalar(
                    out=amin_row, in0=amin_row, scalar1=float(STEP_ALPHA),
                    scalar2=0.995, op0=OP.min, op1=OP.mult,
                )
                a8_ps = ms_ps.tile([2, 1], F32, tag="ms")
                nc.tensor.transpose(a8_ps, amin_row, identf[0:1, 0:1])
                a8 = smt.tile([2, 1], F32, tag="a8")
                nc.scalar.copy(a8, a8_ps)
                acol_ps = ms_ps.tile([128, 1], F32, tag="ms")
                nc.tensor.matmul(
                    acol_ps[pr, :], seg2, a8, start=True, stop=True,
                    tile_position=(0, 32 * Q),
                )
                acol = smt.tile([128, 1], F32, tag="acol")
                nc.vector.tensor_copy(acol[pr, :], acol_ps[pr, :])

                ay = smt.tile([128, 128], F32, tag="ay", bufs=1)
                nc.vector.tensor_scalar(
                    out=ay[pr, :], in0=y_pm[pr, :], scalar1=acol[pr, :],
                    scalar2=None, op0=OP.mult,
                )
                nc.vector.tensor_add(xs[pr, :], xs[pr, :], ay[pr, :])

            # ---------------- pipelined slot loop ----------------
            df_cols = smt.tile([128, GPC, FCH], BF16, tag="df_cols")
            y_pmA = smt.tile([128, 128], BF16, tag="y_pmA")
            y_pmB = smt.tile([128, 128], BF16, tag="y_pmB")
            preds_live = {}  # (s, Q) -> pred tile awaiting its tail

            for t in range(num_steps + 1):
                for Q in range(NQ):
                    tq = Q - 2
                    ts_ = t
                    if tq < 0:
                        tq += NQ
                        ts_ = t - 1
                    if 0 <= ts_ <= num_steps - 1 and (ts_, tq) in preds_live:
                        emit_tail(ts_, tq, preds_live.pop((ts_, tq)))
                    if 1 <= t <= num_steps - 1:
                        emit_e2ls(t - 1, Q)
                    if t <= num_steps - 1:
                        preds_live[(t, Q)] = emit_mlp(t, Q)

    _split_sync_waits(nc, maxw=1)
    return nc


def _seg_mats():
    seg = np.zeros((128, 128), np.float32)
    for g in range(GPC):
        seg[16 * g : 16 * g + 16, 16 * g : 16 * g + 16] = 1.0
    seg2 = np.zeros((2, 32), np.float32)
    for gl in range(2):
        seg2[gl, 16 * gl : 16 * gl + 16] = 1.0
    return seg, seg2


def _prep_core_inputs(core, proj, x_start, x_solution, node_feat, W1, b1, W2, b2):
    g0 = core * GPC
    n0 = core * NPC
    Pc = proj[g0 : g0 + GPC]  # [8, 2048, 512] f32
    P_bf = np.ascontiguousarray(
        Pc.reshape(GPC, NCH, 128, F).transpose(2, 0, 1, 3)
    ).astype(BF)
    PT_bf = np.ascontiguousarray(Pc.transpose(0, 2, 1)).reshape(
        GPC, FCH, 128, NMAX
    ).astype(BF)
    nfT = np.ascontiguousarray(node_feat[n0 : n0 + NPC].T).astype(BF)
    seg, seg2 = _seg_mats()
    return {
        "P": P_bf,
        "PT": PT_bf,
        "nfT": nfT,
        "xs0": x_start[n0 : n0 + NPC].reshape(128, 128).astype(np.float32),
        "xsol": x_solution[n0 : n0 + NPC].reshape(128, 128).astype(np.float32),
        "w1": W1.astype(BF),
        "b1": b1.reshape(HID, 1).astype(np.float32),
        "w2": W2.reshape(HID, 1).astype(BF),
        "b2": b2.reshape(1, 1).astype(np.float32),
        "seg": seg,
        "seg2": seg2,
    }


def _numpy_fallback(x_start, x_solution, node_feat, proj_matrix, W1, b1, W2, b2, batch):
    """General (ragged) reference implementation in numpy, used only if
    vals_batch is not the expected equal-size pattern."""
    nb = proj_matrix.shape[0]
    batch = batch.astype(np.int64)
    counts = np.bincount(batch, minlength=nb)
    offsets = np.cumsum(counts) - counts
    pos = np.arange(batch.shape[0]) - offsets[batch]

    def l1norm(x):
        s = np.zeros(nb, x.dtype)
        np.add.at(s, batch, np.abs(x))
        return x / np.clip(s, 1e-8, None)[batch]

    def to_dense(x):
        dense = np.zeros((nb, NMAX), x.dtype)
        m = pos < NMAX
        dense[batch[m], pos[m]] = x[m]
        return dense

    def line_search(x, dvec):
        neg = dvec < 0
        step = np.where(neg, x / np.where(neg, -dvec, 1.0), STEP_ALPHA)
        a = np.full(nb, np.inf, step.dtype)
        np.minimum.at(a, batch, step)
        return np.minimum(a, STEP_ALPHA)[batch]

    def gnn(x):
        h = np.concatenate([node_feat, x[:, None]], axis=-1)
        h = np.maximum(h @ W1 + b1, 0.0)
        return (h @ W2 + b2)[:, 0]

    tau = 0.01
    xs = x_start.astype(np.float32)
    preds, labels = [], []
    for _ in range(NUM_STEPS):
        pred = gnn(xs)
        preds.append(pred)
        labels.append(l1norm(x_solution - xs))
        p = l1norm(pred)
        direction = p + 3.0 * tau / (xs + tau)
        tau = max(tau * 0.5, 1e-5)
        d_dense = to_dense(direction)
        df = np.einsum("bnf,bn->bf", proj_matrix, d_dense)
        proj_dense = np.einsum("bnf,bf->bn", proj_matrix, df)
        proj_flat = proj_dense[batch, np.minimum(pos, NMAX - 1)]
        proj_flat = np.where(pos < NMAX, proj_flat, 0.0)
        alpha = line_search(xs, proj_flat) * 0.995
        xs = xs + alpha * proj_flat
    return np.stack(preds, 1).astype(np.float32), np.stack(labels, 1).astype(np.float32)


def run_on_hw(inputs_list, debug=False):
    key = "plain"
    if key not in _COMPILED:
        _COMPILED[key] = build_nc()
    nc = _COMPILED[key]
    return run_bass_kernel_spmd(nc, inputs_list, list(range(NCORES))).results


def kernel(x_start, x_solution, node_feat, proj_matrix, W1, b1, W2, b2, vals_batch):
    expected = np.repeat(np.arange(B, dtype=np.int64), NMAX)
    vb = np.asarray(vals_batch)
    if vb.shape != expected.shape or not np.array_equal(
        vb.astype(np.int64), expected
    ):
        return _numpy_fallback(
            np.asarray(x_start, np.float32),
            np.asarray(x_solution, np.float32),
            np.asarray(node_feat, np.float32),
            np.asarray(proj_matrix, np.float32),
            np.asarray(W1, np.float32),
            np.asarray(b1, np.float32),
            np.asarray(W2, np.float32),
            np.asarray(b2, np.float32),
            vb,
        )

    x_start = np.asarray(x_start, np.float32)
    x_solution = np.asarray(x_solution, np.float32)
    node_feat = np.asarray(node_feat, np.float32)
    proj_matrix = np.asarray(proj_matrix, np.float32)
    W1 = np.asarray(W1, np.float32)
    b1 = np.asarray(b1, np.float32)
    W2 = np.asarray(W2, np.float32)
    b2 = np.asarray(b2, np.float32)

    ins = [
        _prep_core_inputs(c, proj_matrix, x_start, x_solution, node_feat, W1, b1, W2, b2)
        for c in range(NCORES)
    ]
    results = run_on_hw(ins)
    preds = np.concatenate(
        [results[c]["preds"].T for c in range(NCORES)], axis=0
    ).astype(np.float32)
    labels = np.concatenate(
        [results[c]["labels"].T for c in range(NCORES)], axis=0
    ).astype(np.float32)
    return preds, labels
